# revision 12
# baseline (speedup 1.0000x reference)
"""DecoderAttention (GQA + RoPE + causal) Trainium2 Bass kernel.

Sharding over 8 NeuronCores: core = 4*batch + g where g in [0,4) is the
head-group. Each core computes 4 query heads (o-slice 512g:512g+512 of Wq)
and their shared KV head (slice 128g:128g+128 of Wk/Wv), plus the partial
output projection with the matching 512-column slice of Wo. Host sums the 4
partials per batch.

Per-core dataflow (matmul inputs bf16, f32 PSUM accumulate):
  QT[o,t] = WqT.T @ hsT   (transposed projections; hsT streamed once)
  RoPE applied in [d,t] layout via partition-offset DVE ops (bf16)
  ST[k,q] = KT_tile.T @ QT  -> exp on ACT (scale folded) -> P[k,q]
  attn_outT[d,q] += V_tile.T @ P ; rowsum[1,q] += ones.T @ P
  normalize at the PSUM->SBUF copy; out[t,h] += ao_tile.T @ WoT
Causal structure: fully-masked k-tiles skipped; diagonal-tile score matmuls,
exp, rowsum and AV matmuls are all column-sliced to the valid q range.
"""
import math
import os
import sys

sys.path.insert(0, "/opt/trn_rl_repo")

import numpy as np
import ml_dtypes

import concourse.bass as bass  # noqa: F401  (registers engines)
import concourse.mybir as mybir
import concourse.tile as tile
from concourse import bacc
from concourse.bass_utils import run_bass_kernel_spmd

B, T, HID = 2, 2048, 2048
H, KVH, D = 16, 4, 128
NH = H // KVH          # q-heads per core = 4
TM = 512               # t/q macro tile
NKT = HID // 128       # 16 contraction k-tiles for projections
NTT = T // 128         # 16 t-tiles
NM = T // TM           # 4 macros
SCALE = 1.0 / math.sqrt(D)
NEG = -1.0e30

f32 = mybir.dt.float32
bf16 = mybir.dt.bfloat16
fp8 = mybir.dt.float8e4
NP_IN = ml_dtypes.bfloat16
NP_F8 = ml_dtypes.float8_e4m3
DR = mybir.MatmulPerfMode.DoubleRow
WS = 32.0               # host pre-scale on Wq/Wk/Wv for the fp8 hi/lo split
FP8MAX = 240.0
EXP = mybir.ActivationFunctionType.Exp
IDENT = mybir.ActivationFunctionType.Identity
MULT = mybir.AluOpType.mult
ADD = mybir.AluOpType.add

LAST_RESULTS = None  # BassKernelResults of the most recent run (for test.py)

_cache = {}


def _emit(nc, tc, causal):
    ap = {}
    # hi/lo fp8 split operands, host-interleaved as [kpair, 128, 2, cols] so a
    # [128, 2, N] DoubleRow tile is a single DMA
    ap["hsTh"] = nc.dram_tensor("hsTh", [8, 128, 2, T], fp8, kind="ExternalInput").ap()
    ap["hsTl"] = nc.dram_tensor("hsTl", [8, 128, 2, T], fp8, kind="ExternalInput").ap()
    ap["wqkvh"] = nc.dram_tensor("wqkvh", [8, 128, 2, 768], fp8, kind="ExternalInput").ap()
    ap["wqkvl"] = nc.dram_tensor("wqkvl", [8, 128, 2, 768], fp8, kind="ExternalInput").ap()
    ap["woT"] = nc.dram_tensor("woT", [512, HID], bf16, kind="ExternalInput").ap()
    ap["bias"] = nc.dram_tensor("bias", [128, 6], f32, kind="ExternalInput").ap()
    ap["cosT"] = nc.dram_tensor("cosT", [D, T], bf16, kind="ExternalInput").ap()
    ap["sinTs"] = nc.dram_tensor("sinTs", [D, T], bf16, kind="ExternalInput").ap()
    ap["dmask"] = nc.dram_tensor("dmask", [4, 128, 128], bf16, kind="ExternalInput").ap()
    ap["ones1"] = nc.dram_tensor("ones1", [128, 1], bf16, kind="ExternalInput").ap()
    ap["ident"] = nc.dram_tensor("ident", [128, 128], bf16, kind="ExternalInput").ap()
    if not causal:
        ap["maskT"] = nc.dram_tensor("maskT", [T, T], f32, kind="ExternalInput").ap()
    out_part = nc.dram_tensor("out_part", [T, HID], bf16, kind="ExternalOutput").ap()

    with tc.tile_pool(name="persist", bufs=1) as pper, \
         tc.tile_pool(name="wqkv", bufs=1) as pw, \
         tc.tile_pool(name="wo", bufs=1) as pwo, \
         tc.tile_pool(name="ropecs", bufs=1) as pcs, \
         tc.tile_pool(name="phA", bufs=2) as pa, \
         tc.tile_pool(name="hst", bufs=16) as ph, \
         tc.tile_pool(name="ptile", bufs=18) as pp, \
         tc.tile_pool(name="phB", bufs=2) as pb, \
         tc.tile_pool(name="mask", bufs=3) as pm, \
         tc.tile_pool(name="outp", bufs=4) as po:
        qt = [pper.tile([128, T], bf16, tag=f"qt{h}", name=f"qt{h}") for h in range(NH)]
        kt = pper.tile([128, T], bf16, tag="kt", name="kt")
        vsb = pper.tile([128, T], bf16, tag="vsb", name="vsb")
        ao = [pper.tile([128, T], bf16, tag=f"ao{h}", name=f"ao{h}") for h in range(NH)]
        # Matmul-critical DMAs first, interleaved across the sync and gpsimd
        # queues so the first macro's hsT tiles are all prefetched without
        # gating on PE progress. Small constant loads go afterwards.
        wh_tiles, wl_tiles = [], []
        pre_ht = []
        for kp in range(8):
            wh = pw.tile([128, 2, 768], fp8, tag=f"wh{kp}", name=f"wh{kp}")
            nc.sync.dma_start(out=wh[:], in_=ap["wqkvh"][kp])
            wl = pw.tile([128, 2, 768], fp8, tag=f"wl{kp}", name=f"wl{kp}")
            nc.gpsimd.dma_start(out=wl[:], in_=ap["wqkvl"][kp])
            wh_tiles.append(wh)
            wl_tiles.append(wl)
            xh = ph.tile([128, 2, TM], fp8, tag="xh", name="xh")
            nc.sync.dma_start(out=xh[:], in_=ap["hsTh"][kp, :, :, 0:TM])
            xl = ph.tile([128, 2, TM], fp8, tag="xl", name="xl")
            nc.gpsimd.dma_start(out=xl[:], in_=ap["hsTl"][kp, :, :, 0:TM])
            pre_ht.append((xh, xl))
        bias_t = pper.tile([128, 6], f32, tag="bias", name="bias")
        nc.gpsimd.dma_start(out=bias_t[:], in_=ap["bias"][:])
        ones_r = pper.tile([128, 1], bf16, tag="ones1", name="ones1")
        nc.gpsimd.dma_start(out=ones_r[:], in_=ap["ones1"][:])
        ident_t = pper.tile([128, 128], bf16, tag="ident", name="ident")
        nc.gpsimd.dma_start(out=ident_t[:], in_=ap["ident"][:])
        dmask_t = pper.tile([128, 4 * 128], bf16, tag="dmask", name="dmask")
        for j in range(4):
            nc.gpsimd.dma_start(
                out=dmask_t[:, 128 * j:128 * (j + 1)], in_=ap["dmask"][j])
        cos_t = pcs.tile([128, T], bf16, tag="cosT", name="cosT")
        nc.gpsimd.dma_start(out=cos_t[:], in_=ap["cosT"][:])
        sins_t = pcs.tile([128, T], bf16, tag="sinTs", name="sinTs")
        nc.gpsimd.dma_start(out=sins_t[:], in_=ap["sinTs"][:])
        wo_tiles = [pwo.tile([128, HID], bf16, tag=f"wo{o}", name=f"wo{o}")
                    for o in range(4)]

        def load_wo():
            for o in range(4):
                nc.sync.dma_start(
                    out=wo_tiles[o][:], in_=ap["woT"][128 * o:128 * (o + 1), :])

        # ---------------- Phase A: projections + RoPE + V transpose ---------
        with tc.tile_pool(name="psA", bufs=1, space="PSUM") as psa, \
             tc.tile_pool(name="psAtr", bufs=2, space="PSUM") as psatr:

            def rope(dst, src, tsl):
                # dst = src*cos + rotate_half(src)*sin, in [d, t] layout, bf16.
                # sins_t rows d<64 hold +sin[d+64], rows d>=64 hold -sin[d-64],
                # so each mul reads both SBUF inputs at the same base partition
                # (walrus requires equal input base partitions); only the
                # output is partition-shifted.
                tmp = pa.tile([128, TM], bf16, tag="ropetmp", name="ropetmp")
                nc.vector.tensor_tensor(
                    out=tmp[0:64, :], in0=src[64:128, :], in1=sins_t[64:128, tsl], op=MULT)
                nc.vector.tensor_tensor(
                    out=tmp[64:128, :], in0=src[0:64, :], in1=sins_t[0:64, tsl], op=MULT)
                tmp2 = pa.tile([128, TM], bf16, tag="ropetmp2", name="ropetmp2")
                nc.vector.tensor_tensor(
                    out=tmp2[:], in0=src[:], in1=cos_t[:, tsl], op=MULT)
                nc.gpsimd.tensor_tensor(out=dst, in0=tmp2[:], in1=tmp[:], op=ADD)

            vraw_prev = None

            def v_transposes(m, vraw):
                for j in range(4):
                    tt = 4 * m + j
                    tr_ps = psatr.tile([128, 128], bf16, tag="vtr", name="vtr")
                    nc.tensor.transpose(
                        tr_ps[:], vraw[:, 128 * j:128 * (j + 1)], ident_t[:])
                    nc.scalar.copy(vsb[:, 128 * tt:128 * (tt + 1)], tr_ps[:])

            for m in range(NM):
                tsl = slice(TM * m, TM * (m + 1))
                q_ps = [psa.tile([128, TM], f32, tag=f"psq{o}", name=f"psq{o}")
                        for o in range(NH)]
                k_ps = psa.tile([128, TM], f32, tag="psk", name="psk")
                v_ps = psa.tile([128, TM], f32, tag="psv", name="psv")
                outs = q_ps + [k_ps, v_ps]
                for kp in range(8):
                    if m == 0:
                        xh, xl = pre_ht[kp]
                    else:
                        xh = ph.tile([128, 2, TM], fp8, tag="xh", name="xh")
                        nc.sync.dma_start(out=xh[:], in_=ap["hsTh"][kp, :, :, tsl])
                        xl = ph.tile([128, 2, TM], fp8, tag="xl", name="xl")
                        nc.gpsimd.dma_start(out=xl[:], in_=ap["hsTl"][kp, :, :, tsl])
                    st = (kp == 0)
                    sp = (kp == 7)
                    for o in range(6):
                        osl = slice(128 * o, 128 * (o + 1))
                        # hi*hi + lo*hi + hi*lo, one f32 accumulation group
                        nc.tensor.matmul(
                            outs[o][:], wh_tiles[kp][:, :, osl], xh[:],
                            start=st, stop=False, perf_mode=DR)
                        nc.tensor.matmul(
                            outs[o][:], wh_tiles[kp][:, :, osl], xl[:],
                            start=False, stop=False, perf_mode=DR)
                        nc.tensor.matmul(
                            outs[o][:], wl_tiles[kp][:, :, osl], xh[:],
                            start=False, stop=sp, perf_mode=DR)
                # previous macro's V transposes: their inputs are long since
                # ready, so they never stall the PE at the macro boundary.
                if vraw_prev is not None:
                    v_transposes(m - 1, vraw_prev)
                # drain the six accumulators on ACT (idle during phase A,
                # keeping DVE/Pool free for the rope chain)
                raws = []
                for o in range(NH):
                    qraw = pa.tile([128, TM], bf16, tag=f"qraw{o}", name=f"qraw{o}")
                    nc.scalar.activation(
                        qraw[:], q_ps[o][:], IDENT, bias=bias_t[:, o:o + 1])
                    raws.append(qraw)
                kraw = pa.tile([128, TM], bf16, tag="kraw", name="kraw")
                nc.scalar.activation(kraw[:], k_ps[:], IDENT, bias=bias_t[:, 4:5])
                for o in range(NH):
                    rope(qt[o][:, tsl], raws[o], tsl)
                rope(kt[:, tsl], kraw, tsl)
                vraw = pa.tile([128, TM], bf16, tag="vraw", name="vraw", bufs=2)
                nc.scalar.activation(vraw[:], v_ps[:], IDENT, bias=bias_t[:, 5:6])
                vraw_prev = vraw
                if m == 1:
                    load_wo()
            v_transposes(NM - 1, vraw_prev)

        # ---------------- Phase B + C: attention + output projection --------
        with tc.tile_pool(name="psSC", bufs=2, space="PSUM") as ps_sc, \
             tc.tile_pool(name="psAV", bufs=1, space="PSUM") as ps_av, \
             tc.tile_pool(name="psRS", bufs=1, space="PSUM") as ps_rs:
            otidx = 0
            for m in (1, 2, 3, 0):
                nk = 4 * (m + 1) if causal else NTT
                qsl = slice(TM * m, TM * (m + 1))
                for pair in range(NH // 2):
                    h0, h1 = 2 * pair, 2 * pair + 1
                    av = [ps_av.tile([128, TM], f32, tag=f"av{i}", name=f"av{i}")
                          for i in range(2)]
                    rs = [ps_rs.tile([1, TM], f32, tag=f"rs{i}", name=f"rs{i}")
                          for i in range(2)]
                    # Software pipeline: scores+exp for all visits first; then
                    # rowsums; the reciprocal/broadcast chain overlaps the AV
                    # matmuls; each head's normalize overlaps the other head's
                    # AV streams.
                    pt_q = []

                    def _q0(kk):
                        # first q column this visit contributes to (causal):
                        # q_local < 128*jp is entirely masked, never read
                        jp = kk - (nk - 4)
                        return 128 * jp if (causal and jp > 0) else 0

                    def rs_mms(kk, pt):
                        q0 = _q0(kk)
                        st = (kk == 0)
                        sp = (kk == nk - 1)
                        nc.tensor.matmul(rs[0][:, q0:TM], ones_r[:],
                                         pt[:, 0, q0:TM], start=st, stop=sp)
                        nc.tensor.matmul(rs[1][:, q0:TM], ones_r[:],
                                         pt[:, 1, q0:TM], start=st, stop=sp)

                    def av_mms(kk, pt, i):
                        q0 = _q0(kk)
                        ksl = slice(128 * kk, 128 * (kk + 1))
                        st = (kk == 0)
                        sp = (kk == nk - 1)
                        nc.tensor.matmul(av[i][:, q0:TM], vsb[:, ksl],
                                         pt[:, i, q0:TM], start=st, stop=sp)

                    for kk in range(nk):
                        ksl = slice(128 * kk, 128 * (kk + 1))
                        q0 = _q0(kk)
                        # both heads' score tiles side by side -> one exp pass
                        sc = ps_sc.tile([128, 2, TM], f32, tag="sc", name="sc")
                        nc.tensor.matmul(sc[:, 0, q0:TM], kt[:, ksl],
                                         qt[h0][:, TM * m + q0:TM * (m + 1)],
                                         start=True, stop=True)
                        nc.tensor.matmul(sc[:, 1, q0:TM], kt[:, ksl],
                                         qt[h1][:, TM * m + q0:TM * (m + 1)],
                                         start=True, stop=True)
                        pt = pp.tile([128, 2, TM], bf16, tag="pt", name="pt")
                        if causal:
                            nc.scalar.activation(
                                pt[:, :, q0:TM], sc[:, :, q0:TM], EXP,
                                scale=SCALE / (WS * WS))
                            jp = kk - (nk - 4)
                            if jp >= 0:
                                # mask after exp: only the 128x128 diagonal
                                # triangle is ever read partially masked (the
                                # region left of it is skipped by the sliced
                                # AV/rowsum matmuls)
                                w0, w1 = 128 * jp, 128 * (jp + 1)
                                for i in range(2):
                                    nc.vector.tensor_tensor(
                                        out=pt[:, i, w0:w1],
                                        in0=pt[:, i, w0:w1],
                                        in1=dmask_t[:, 128 * jp:128 * (jp + 1)],
                                        op=MULT)
                        else:
                            mk = pm.tile([128, TM], f32, tag="mk", name="mk")
                            nc.sync.dma_start(
                                out=mk[:], in_=ap["maskT"][ksl, qsl])
                            for i in range(2):
                                nc.vector.scalar_tensor_tensor(
                                    out=sc[:, i, :],
                                    in0=sc[:, i, :],
                                    scalar=SCALE / (WS * WS), in1=mk[:],
                                    op0=MULT, op1=ADD)
                            nc.scalar.activation(pt[:], sc[:], EXP, scale=1.0)
                        pt_q.append((kk, pt))
                    # rowsum matmuls first so the reciprocal/broadcast chain
                    # runs concurrently with the AV matmuls.
                    for kk2, pt2 in pt_q:
                        rs_mms(kk2, pt2)
                    invbs = []
                    for i in range(2):
                        inv = pb.tile([1, TM], f32, tag="inv", name="inv")
                        nc.vector.reciprocal_approx_fast(out=inv[:], in_=rs[i][:])
                        invb = pb.tile([128, TM], f32, tag=f"invb{i}",
                                       name=f"invb{i}")
                        nc.gpsimd.partition_broadcast(invb[:], inv[:])
                        invbs.append(invb)
                    # head 0's AV streams, then its normalize overlaps head 1's
                    # AV streams, so only normalize(h1) sits on the critical
                    # path into phase C.
                    for i, h in ((0, h0), (1, h1)):
                        for kk2, pt2 in pt_q:
                            av_mms(kk2, pt2, i)
                        nc.vector.tensor_tensor(
                            out=ao[h][:, qsl], in0=av[i][:], in1=invbs[i][:],
                            op=MULT)
                    pt_q.clear()
                # Phase C for the 4 t-tiles of this macro
                for j in range(4):
                    tt = 4 * m + j
                    ttsl = slice(128 * tt, 128 * (tt + 1))
                    for hc in range(4):
                        hsl = slice(512 * hc, 512 * (hc + 1))
                        idx = (4 * j + hc) % 4
                        if idx < 2:
                            op_ps = ps_av.tile([128, TM], f32, tag=f"av{idx}",
                                               name="opps")
                        else:
                            op_ps = ps_rs.tile([128, TM], f32, tag=f"rs{idx - 2}",
                                               name="opps")
                        for o in range(4):
                            nc.tensor.matmul(
                                op_ps[:], ao[o][:, ttsl], wo_tiles[o][:, hsl],
                                start=(o == 0), stop=(o == 3))
                        ot = po.tile([128, TM], bf16, tag="ot", name="ot")
                        otidx += 1
                        if otidx % 2 == 0:
                            nc.scalar.copy(ot[:], op_ps[:])
                        else:
                            nc.vector.tensor_copy(ot[:], op_ps[:])
                        nc.sync.dma_start(out=out_part[ttsl, hsl], in_=ot[:])


def _build(causal):
    nc = bacc.Bacc("TRN2", target_bir_lowering=False, debug=False, num_devices=8)
    with tile.TileContext(nc) as tc:
        _emit(nc, tc, causal)
    nc.compile()
    return nc


def _canonical_causal_mask():
    neg = np.float32(np.finfo(np.float32).min)
    m = np.where(np.tril(np.ones((T, T), dtype=bool)), np.float32(0.0), neg)
    return m.astype(np.float32)


def kernel(**inputs):
    global LAST_RESULTS
    hs = np.ascontiguousarray(np.asarray(inputs["hidden_states"], dtype=np.float32))
    cos = np.asarray(inputs["cos"], dtype=np.float32)
    sin = np.asarray(inputs["sin"], dtype=np.float32)
    mask = np.asarray(inputs["attention_mask"], dtype=np.float32)
    Wq = np.asarray(inputs["Wq"], dtype=np.float32)
    Wk = np.asarray(inputs["Wk"], dtype=np.float32)
    Wv = np.asarray(inputs["Wv"], dtype=np.float32)
    Wo = np.asarray(inputs["Wo"], dtype=np.float32)
    bq = np.asarray(inputs["bq"], dtype=np.float32)
    bk = np.asarray(inputs["bk"], dtype=np.float32)
    bv = np.asarray(inputs["bv"], dtype=np.float32)

    causal = bool(np.array_equal(mask[0, 0], _canonical_causal_mask()))

    key = (causal,)
    if key not in _cache:
        _cache[key] = _build(causal)
    nc = _cache[key]

    tri01 = (np.arange(128)[:, None] <= np.arange(128)[None, :])
    dmask = np.broadcast_to(tri01.astype(NP_IN), (4, 128, 128)).copy()
    ident = np.eye(128, dtype=NP_IN)
    ones1 = np.ones((128, 1), dtype=NP_IN)
    if not causal:
        maskT = np.ascontiguousarray(mask[0, 0].T)

    def q8(x):
        return np.clip(x, -FP8MAX, FP8MAX).astype(NP_F8)

    def interleave(x):
        # [rows, cols] f32 -> hi/lo fp8 pair, each [rows/256, 128, 2, cols]
        hi = q8(x)
        lo = q8(x - hi.astype(np.float32))
        out = []
        for a in (hi, lo):
            r = a.reshape(-1, 2, 128, a.shape[-1]).transpose(0, 2, 1, 3)
            out.append(np.ascontiguousarray(r))
        return out

    in_maps = []
    for c in range(8):
        b, g = divmod(c, 4)
        sl_q = slice(512 * g, 512 * (g + 1))
        sl_kv = slice(128 * g, 128 * (g + 1))
        sinT = np.ascontiguousarray(sin[b].T)  # [D, T]
        # row d<64: +sin[d+64] (consumed at base partition 0 writing rows 64:128)
        # row d>=64: -sin[d-64] (consumed at base partition 64 writing rows 0:64)
        sinTs = np.concatenate([sinT[64:128], -sinT[0:64]], axis=0)
        bias = np.zeros((128, 6), dtype=np.float32)
        bias[:, 0:4] = bq[sl_q].reshape(4, 128).T
        bias[:, 4] = bk[sl_kv]
        bias[:, 5] = bv[sl_kv]
        bias *= WS  # projection outputs carry the WS weight pre-scale
        hsTh, hsTl = interleave(hs[b].T)
        wqkvh, wqkvl = interleave(
            WS * np.concatenate([Wq[sl_q], Wk[sl_kv], Wv[sl_kv]], axis=0).T)
        m = {
            "hsTh": hsTh,
            "hsTl": hsTl,
            "wqkvh": wqkvh,
            "wqkvl": wqkvl,
            # ao carries the WS scale from the projections; fold 1/WS into Wo
            "woT": np.ascontiguousarray((Wo[:, sl_q].T / WS).astype(NP_IN)),
            "bias": bias,
            "cosT": np.ascontiguousarray(cos[b].T.astype(NP_IN)),
            "sinTs": np.ascontiguousarray(sinTs.astype(NP_IN)),
            "dmask": dmask,
            "ones1": ones1,
            "ident": ident,
        }
        if not causal:
            m["maskT"] = maskT
        in_maps.append(m)

    trace = os.environ.get("KERNEL_TRACE", "0") == "1"
    res = run_bass_kernel_spmd(nc, in_maps, list(range(8)), trace=trace)
    LAST_RESULTS = res

    out = np.empty((B, T, HID), dtype=np.float32)
    for b in range(B):
        acc = res.results[4 * b]["out_part"].astype(np.float32)
        for g in range(1, 4):
            acc += res.results[4 * b + g]["out_part"].astype(np.float32)
        out[b] = acc
    return out


# revision 20
# speedup vs baseline: 1.1172x; 1.1172x over previous
"""DecoderAttention (GQA + RoPE + causal) Trainium2 Bass kernel.

Sharding over 8 NeuronCores: core = 4*batch + g where g in [0,4) is the
head-group. Each core computes 4 query heads (o-slice 512g:512g+512 of Wq)
and their shared KV head (slice 128g:128g+128 of Wk/Wv), plus the partial
output projection with the matching 512-column slice of Wo. Host sums the 4
partials per batch.

Per-core dataflow (matmul inputs bf16, f32 PSUM accumulate):
  QT[o,t] = WqT.T @ hsT   (transposed projections; hsT streamed once)
  RoPE applied in [d,t] layout via partition-offset DVE ops (bf16)
  ST[k,q] = KT_tile.T @ QT  -> exp on ACT (scale folded) -> P[k,q]
  attn_outT[d,q] += V_tile.T @ P ; rowsum[1,q] += ones.T @ P
  normalize at the PSUM->SBUF copy; out[t,h] += ao_tile.T @ WoT
Causal structure: fully-masked k-tiles skipped; diagonal-tile score matmuls,
exp, rowsum and AV matmuls are all column-sliced to the valid q range.
"""
import math
import os
import sys

sys.path.insert(0, "/opt/trn_rl_repo")

import numpy as np
import ml_dtypes

import concourse.bass as bass  # noqa: F401  (registers engines)
import concourse.mybir as mybir
import concourse.tile as tile
from concourse import bacc
from concourse.bass_utils import run_bass_kernel_spmd

B, T, HID = 2, 2048, 2048
H, KVH, D = 16, 4, 128
NH = H // KVH          # q-heads per core = 4
TM = 512               # t/q macro tile
NKT = HID // 128       # 16 contraction k-tiles for projections
NTT = T // 128         # 16 t-tiles
NM = T // TM           # 4 macros
SCALE = 1.0 / math.sqrt(D)
NEG = -1.0e30

f32 = mybir.dt.float32
bf16 = mybir.dt.bfloat16
fp8 = mybir.dt.float8e4
NP_IN = ml_dtypes.bfloat16
NP_F8 = ml_dtypes.float8_e4m3
DR = mybir.MatmulPerfMode.DoubleRow
WS = 32.0               # host pre-scale on Wq/Wk/Wv for the fp8 hi/lo split
FP8MAX = 240.0
EXP = mybir.ActivationFunctionType.Exp
IDENT = mybir.ActivationFunctionType.Identity
MULT = mybir.AluOpType.mult
ADD = mybir.AluOpType.add

LAST_RESULTS = None  # BassKernelResults of the most recent run (for test.py)

_cache = {}


def _emit(nc, tc, causal):
    ap = {}
    ap["hsT"] = nc.dram_tensor("hsT", [HID, T], bf16, kind="ExternalInput").ap()
    ap["wqkvT"] = nc.dram_tensor("wqkvT", [HID, 768], bf16, kind="ExternalInput").ap()
    ap["woT"] = nc.dram_tensor("woT", [512, HID], bf16, kind="ExternalInput").ap()
    ap["bias"] = nc.dram_tensor("bias", [128, 6], f32, kind="ExternalInput").ap()
    ap["cosT"] = nc.dram_tensor("cosT", [D, T], bf16, kind="ExternalInput").ap()
    ap["sinTs"] = nc.dram_tensor("sinTs", [D, T], bf16, kind="ExternalInput").ap()
    ap["dmask"] = nc.dram_tensor("dmask", [4, 128, 128], bf16, kind="ExternalInput").ap()
    ap["ones1"] = nc.dram_tensor("ones1", [128, 1], bf16, kind="ExternalInput").ap()
    ap["ident"] = nc.dram_tensor("ident", [128, 128], bf16, kind="ExternalInput").ap()
    if not causal:
        ap["maskT"] = nc.dram_tensor("maskT", [T, T], f32, kind="ExternalInput").ap()
    out_part = nc.dram_tensor("out_part", [T, HID], bf16, kind="ExternalOutput").ap()

    with tc.tile_pool(name="persist", bufs=1) as pper, \
         tc.tile_pool(name="wqkv", bufs=1) as pw, \
         tc.tile_pool(name="wo", bufs=1) as pwo, \
         tc.tile_pool(name="ropecs", bufs=1) as pcs, \
         tc.tile_pool(name="phA", bufs=2) as pa, \
         tc.tile_pool(name="hst", bufs=16) as ph, \
         tc.tile_pool(name="ptile", bufs=18) as pp, \
         tc.tile_pool(name="phB", bufs=2) as pb, \
         tc.tile_pool(name="mask", bufs=3) as pm, \
         tc.tile_pool(name="outp", bufs=4) as po:
        qt = [pper.tile([128, T], bf16, tag=f"qt{h}", name=f"qt{h}") for h in range(NH)]
        kt = pper.tile([128, T], bf16, tag="kt", name="kt")
        vsb = pper.tile([128, T], bf16, tag="vsb", name="vsb")
        ao = [pper.tile([128, T], bf16, tag=f"ao{h}", name=f"ao{h}") for h in range(NH)]
        # Matmul-critical DMAs first, interleaved across the sync and gpsimd
        # queues so the first macro's hsT tiles are all prefetched without
        # gating on PE progress. Small constant loads go afterwards.
        w_tiles = []
        pre_ht = []
        for k in range(NKT):
            wt = pw.tile([128, 768], bf16, tag=f"w{k}", name=f"w{k}")
            nc.sync.dma_start(out=wt[:], in_=ap["wqkvT"][128 * k:128 * (k + 1), :])
            w_tiles.append(wt)
            h_t = ph.tile([128, TM], bf16, tag="hst", name="hst")
            eng = nc.gpsimd if (k % 2 == 0) else nc.sync
            eng.dma_start(out=h_t[:], in_=ap["hsT"][128 * k:128 * (k + 1), 0:TM])
            pre_ht.append(h_t)
        bias_t = pper.tile([128, 6], f32, tag="bias", name="bias")
        nc.gpsimd.dma_start(out=bias_t[:], in_=ap["bias"][:])
        ones_r = pper.tile([128, 1], bf16, tag="ones1", name="ones1")
        nc.gpsimd.dma_start(out=ones_r[:], in_=ap["ones1"][:])
        ident_t = pper.tile([128, 128], bf16, tag="ident", name="ident")
        nc.gpsimd.dma_start(out=ident_t[:], in_=ap["ident"][:])
        dmask_t = pper.tile([128, 4 * 128], bf16, tag="dmask", name="dmask")
        for j in range(4):
            nc.gpsimd.dma_start(
                out=dmask_t[:, 128 * j:128 * (j + 1)], in_=ap["dmask"][j])
        cos_t = pcs.tile([128, T], bf16, tag="cosT", name="cosT")
        nc.gpsimd.dma_start(out=cos_t[:], in_=ap["cosT"][:])
        sins_t = pcs.tile([128, T], bf16, tag="sinTs", name="sinTs")
        nc.gpsimd.dma_start(out=sins_t[:], in_=ap["sinTs"][:])
        wo_tiles = [pwo.tile([128, HID], bf16, tag=f"wo{o}", name=f"wo{o}")
                    for o in range(4)]

        def load_wo():
            for o in range(4):
                nc.sync.dma_start(
                    out=wo_tiles[o][:], in_=ap["woT"][128 * o:128 * (o + 1), :])

        # ---------------- Phase A: projections + RoPE + V transpose ---------
        with tc.tile_pool(name="psA", bufs=1, space="PSUM") as psa, \
             tc.tile_pool(name="psAtr", bufs=2, space="PSUM") as psatr:

            def rope(dst, src, tsl):
                # dst = src*cos + rotate_half(src)*sin, in [d, t] layout, bf16.
                # sins_t rows d<64 hold +sin[d+64], rows d>=64 hold -sin[d-64],
                # so each mul reads both SBUF inputs at the same base partition
                # (walrus requires equal input base partitions); only the
                # output is partition-shifted.
                tmp = pa.tile([128, TM], bf16, tag="ropetmp", name="ropetmp")
                nc.vector.tensor_tensor(
                    out=tmp[0:64, :], in0=src[64:128, :], in1=sins_t[64:128, tsl], op=MULT)
                nc.vector.tensor_tensor(
                    out=tmp[64:128, :], in0=src[0:64, :], in1=sins_t[0:64, tsl], op=MULT)
                tmp2 = pa.tile([128, TM], bf16, tag="ropetmp2", name="ropetmp2")
                nc.vector.tensor_tensor(
                    out=tmp2[:], in0=src[:], in1=cos_t[:, tsl], op=MULT)
                nc.gpsimd.tensor_tensor(out=dst, in0=tmp2[:], in1=tmp[:], op=ADD)

            vraw_prev = None

            def v_transposes(m, vraw):
                for j in range(4):
                    tt = 4 * m + j
                    tr_ps = psatr.tile([128, 128], bf16, tag="vtr", name="vtr")
                    nc.tensor.transpose(
                        tr_ps[:], vraw[:, 128 * j:128 * (j + 1)], ident_t[:])
                    nc.scalar.copy(vsb[:, 128 * tt:128 * (tt + 1)], tr_ps[:])

            for m in range(NM):
                tsl = slice(TM * m, TM * (m + 1))
                q_ps = [psa.tile([128, TM], f32, tag=f"psq{o}", name=f"psq{o}")
                        for o in range(NH)]
                k_ps = psa.tile([128, TM], f32, tag="psk", name="psk")
                v_ps = psa.tile([128, TM], f32, tag="psv", name="psv")
                for k in range(NKT):
                    if m == 0:
                        h_t = pre_ht[k]
                    else:
                        h_t = ph.tile([128, TM], bf16, tag="hst", name="hst")
                        eng = nc.gpsimd if (k % 2 == 0) else nc.sync
                        eng.dma_start(
                            out=h_t[:], in_=ap["hsT"][128 * k:128 * (k + 1), tsl])
                    st = (k == 0)
                    sp = (k == NKT - 1)
                    for o in range(NH):
                        nc.tensor.matmul(
                            q_ps[o][:], w_tiles[k][:, 128 * o:128 * (o + 1)], h_t[:],
                            start=st, stop=sp)
                    nc.tensor.matmul(
                        k_ps[:], w_tiles[k][:, 512:640], h_t[:], start=st, stop=sp)
                    nc.tensor.matmul(
                        v_ps[:], w_tiles[k][:, 640:768], h_t[:], start=st, stop=sp)
                # drain the accumulators immediately so the next macro's (or
                # phase B's) PSUM banks free up while the PE runs the previous
                # macro's V transposes; on the last macro split the drains
                # across ACT and DVE to halve the phase-boundary latency.
                last = (m == NM - 1)
                raws = []
                for o in range(NH):
                    qraw = pa.tile([128, TM], bf16, tag=f"qraw{o}", name=f"qraw{o}")
                    if last and o % 2 == 1:
                        nc.vector.tensor_scalar_add(
                            qraw[:], q_ps[o][:], bias_t[:, o:o + 1])
                    else:
                        nc.scalar.activation(
                            qraw[:], q_ps[o][:], IDENT, bias=bias_t[:, o:o + 1])
                    raws.append(qraw)
                kraw = pa.tile([128, TM], bf16, tag="kraw", name="kraw")
                if last:
                    nc.vector.tensor_scalar_add(kraw[:], k_ps[:], bias_t[:, 4:5])
                else:
                    nc.scalar.activation(kraw[:], k_ps[:], IDENT, bias=bias_t[:, 4:5])
                # previous macro's V transposes: their inputs are long since
                # ready, so they never stall the PE at the macro boundary.
                if vraw_prev is not None:
                    v_transposes(m - 1, vraw_prev)
                for o in range(NH):
                    rope(qt[o][:, tsl], raws[o], tsl)
                rope(kt[:, tsl], kraw, tsl)
                vraw = pa.tile([128, TM], bf16, tag="vraw", name="vraw", bufs=2)
                nc.scalar.activation(vraw[:], v_ps[:], IDENT, bias=bias_t[:, 5:6])
                vraw_prev = vraw
                if m == 1:
                    load_wo()
            v_transposes(NM - 1, vraw_prev)

        # ---------------- Phase B + C: attention + output projection --------
        with tc.tile_pool(name="psSC", bufs=2, space="PSUM") as ps_sc, \
             tc.tile_pool(name="psAV", bufs=1, space="PSUM") as ps_av, \
             tc.tile_pool(name="psRS", bufs=1, space="PSUM") as ps_rs:
            otidx = 0
            for m in (1, 2, 3, 0):
                nk = 4 * (m + 1) if causal else NTT
                qsl = slice(TM * m, TM * (m + 1))
                for pair in range(NH // 2):
                    h0, h1 = 2 * pair, 2 * pair + 1
                    av = [ps_av.tile([128, TM], f32, tag=f"av{i}", name=f"av{i}")
                          for i in range(2)]
                    rs = [ps_rs.tile([1, TM], f32, tag=f"rs{i}", name=f"rs{i}")
                          for i in range(2)]
                    # Software pipeline: scores+exp for all visits first; then
                    # rowsums; the reciprocal/broadcast chain overlaps the AV
                    # matmuls; each head's normalize overlaps the other head's
                    # AV streams.
                    pt_q = []

                    def _q0(kk):
                        # first q column this visit contributes to (causal):
                        # q_local < 128*jp is entirely masked, never read
                        jp = kk - (nk - 4)
                        return 128 * jp if (causal and jp > 0) else 0

                    def rs_mms(kk, pt):
                        q0 = _q0(kk)
                        st = (kk == 0)
                        sp = (kk == nk - 1)
                        nc.tensor.matmul(rs[0][:, q0:TM], ones_r[:],
                                         pt[:, 0, q0:TM], start=st, stop=sp)
                        nc.tensor.matmul(rs[1][:, q0:TM], ones_r[:],
                                         pt[:, 1, q0:TM], start=st, stop=sp)

                    def av_mms(kk, pt, i):
                        q0 = _q0(kk)
                        ksl = slice(128 * kk, 128 * (kk + 1))
                        st = (kk == 0)
                        sp = (kk == nk - 1)
                        nc.tensor.matmul(av[i][:, q0:TM], vsb[:, ksl],
                                         pt[:, i, q0:TM], start=st, stop=sp)

                    for kk in range(nk):
                        ksl = slice(128 * kk, 128 * (kk + 1))
                        q0 = _q0(kk)
                        # both heads' score tiles side by side -> one exp pass
                        sc = ps_sc.tile([128, 2, TM], f32, tag="sc", name="sc")
                        nc.tensor.matmul(sc[:, 0, q0:TM], kt[:, ksl],
                                         qt[h0][:, TM * m + q0:TM * (m + 1)],
                                         start=True, stop=True)
                        nc.tensor.matmul(sc[:, 1, q0:TM], kt[:, ksl],
                                         qt[h1][:, TM * m + q0:TM * (m + 1)],
                                         start=True, stop=True)
                        pt = pp.tile([128, 2, TM], bf16, tag="pt", name="pt")
                        if causal:
                            nc.scalar.activation(
                                pt[:, :, q0:TM], sc[:, :, q0:TM], EXP, scale=SCALE)
                            jp = kk - (nk - 4)
                            if jp >= 0:
                                # mask after exp: only the 128x128 diagonal
                                # triangle is ever read partially masked (the
                                # region left of it is skipped by the sliced
                                # AV/rowsum matmuls)
                                w0, w1 = 128 * jp, 128 * (jp + 1)
                                for i in range(2):
                                    nc.vector.tensor_tensor(
                                        out=pt[:, i, w0:w1],
                                        in0=pt[:, i, w0:w1],
                                        in1=dmask_t[:, 128 * jp:128 * (jp + 1)],
                                        op=MULT)
                        else:
                            mk = pm.tile([128, TM], f32, tag="mk", name="mk")
                            nc.sync.dma_start(
                                out=mk[:], in_=ap["maskT"][ksl, qsl])
                            for i in range(2):
                                nc.vector.scalar_tensor_tensor(
                                    out=sc[:, i, :],
                                    in0=sc[:, i, :],
                                    scalar=SCALE, in1=mk[:],
                                    op0=MULT, op1=ADD)
                            nc.scalar.activation(pt[:], sc[:], EXP, scale=1.0)
                        pt_q.append((kk, pt))
                    # rowsum matmuls first so the reciprocal/broadcast chain
                    # runs concurrently with the AV matmuls.
                    for kk2, pt2 in pt_q:
                        rs_mms(kk2, pt2)
                    invbs = []
                    for i in range(2):
                        inv = pb.tile([1, TM], f32, tag="inv", name="inv")
                        nc.vector.reciprocal_approx_fast(out=inv[:], in_=rs[i][:])
                        invb = pb.tile([128, TM], f32, tag=f"invb{i}",
                                       name=f"invb{i}")
                        nc.gpsimd.partition_broadcast(invb[:], inv[:])
                        invbs.append(invb)
                    # head 0's AV streams, then its normalize overlaps head 1's
                    # AV streams, so only normalize(h1) sits on the critical
                    # path into phase C.
                    for i, h in ((0, h0), (1, h1)):
                        for kk2, pt2 in pt_q:
                            av_mms(kk2, pt2, i)
                        nc.vector.tensor_tensor(
                            out=ao[h][:, qsl], in0=av[i][:], in1=invbs[i][:],
                            op=MULT)
                    pt_q.clear()
                # Phase C for the 4 t-tiles of this macro
                for j in range(4):
                    tt = 4 * m + j
                    ttsl = slice(128 * tt, 128 * (tt + 1))
                    # software pipeline: the o<3 partial sums for all 4 output
                    # columns first, so the last head's normalize (which only
                    # lands right before this) is covered by 12 matmuls of
                    # independent work before the o=3 contributions need it.
                    units = []
                    for hc in range(4):
                        hsl = slice(512 * hc, 512 * (hc + 1))
                        if hc < 2:
                            op_ps = ps_av.tile([128, TM], f32, tag=f"av{hc}",
                                               name="opps")
                        else:
                            op_ps = ps_rs.tile([128, TM], f32, tag=f"rs{hc - 2}",
                                               name="opps")
                        for o in range(3):
                            nc.tensor.matmul(
                                op_ps[:], ao[o][:, ttsl], wo_tiles[o][:, hsl],
                                start=(o == 0), stop=False)
                        units.append((op_ps, hsl))
                    for op_ps, hsl in units:
                        nc.tensor.matmul(
                            op_ps[:], ao[3][:, ttsl], wo_tiles[3][:, hsl],
                            start=False, stop=True)
                        ot = po.tile([128, TM], bf16, tag="ot", name="ot")
                        otidx += 1
                        if otidx % 2 == 0:
                            nc.scalar.copy(ot[:], op_ps[:])
                        else:
                            nc.vector.tensor_copy(ot[:], op_ps[:])
                        nc.sync.dma_start(out=out_part[ttsl, hsl], in_=ot[:])


def _build(causal):
    nc = bacc.Bacc("TRN2", target_bir_lowering=False, debug=False, num_devices=8)
    with tile.TileContext(nc) as tc:
        _emit(nc, tc, causal)
    nc.compile()
    return nc


def _canonical_causal_mask():
    neg = np.float32(np.finfo(np.float32).min)
    m = np.where(np.tril(np.ones((T, T), dtype=bool)), np.float32(0.0), neg)
    return m.astype(np.float32)


def kernel(**inputs):
    global LAST_RESULTS
    hs = np.ascontiguousarray(np.asarray(inputs["hidden_states"], dtype=np.float32))
    cos = np.asarray(inputs["cos"], dtype=np.float32)
    sin = np.asarray(inputs["sin"], dtype=np.float32)
    mask = np.asarray(inputs["attention_mask"], dtype=np.float32)
    Wq = np.asarray(inputs["Wq"], dtype=np.float32)
    Wk = np.asarray(inputs["Wk"], dtype=np.float32)
    Wv = np.asarray(inputs["Wv"], dtype=np.float32)
    Wo = np.asarray(inputs["Wo"], dtype=np.float32)
    bq = np.asarray(inputs["bq"], dtype=np.float32)
    bk = np.asarray(inputs["bk"], dtype=np.float32)
    bv = np.asarray(inputs["bv"], dtype=np.float32)

    causal = bool(np.array_equal(mask[0, 0], _canonical_causal_mask()))

    key = (causal,)
    if key not in _cache:
        _cache[key] = _build(causal)
    nc = _cache[key]

    tri01 = (np.arange(128)[:, None] <= np.arange(128)[None, :])
    dmask = np.broadcast_to(tri01.astype(NP_IN), (4, 128, 128)).copy()
    ident = np.eye(128, dtype=NP_IN)
    ones1 = np.ones((128, 1), dtype=NP_IN)
    if not causal:
        maskT = np.ascontiguousarray(mask[0, 0].T)

    in_maps = []
    for c in range(8):
        b, g = divmod(c, 4)
        sl_q = slice(512 * g, 512 * (g + 1))
        sl_kv = slice(128 * g, 128 * (g + 1))
        sinT = np.ascontiguousarray(sin[b].T)  # [D, T]
        # row d<64: +sin[d+64] (consumed at base partition 0 writing rows 64:128)
        # row d>=64: -sin[d-64] (consumed at base partition 64 writing rows 0:64)
        sinTs = np.concatenate([sinT[64:128], -sinT[0:64]], axis=0)
        bias = np.zeros((128, 6), dtype=np.float32)
        bias[:, 0:4] = bq[sl_q].reshape(4, 128).T
        bias[:, 4] = bk[sl_kv]
        bias[:, 5] = bv[sl_kv]
        m = {
            "hsT": np.ascontiguousarray(hs[b].T.astype(NP_IN)),
            "wqkvT": np.ascontiguousarray(
                np.concatenate([Wq[sl_q], Wk[sl_kv], Wv[sl_kv]], axis=0).T.astype(NP_IN)),
            "woT": np.ascontiguousarray(Wo[:, sl_q].T.astype(NP_IN)),
            "bias": bias,
            "cosT": np.ascontiguousarray(cos[b].T.astype(NP_IN)),
            "sinTs": np.ascontiguousarray(sinTs.astype(NP_IN)),
            "dmask": dmask,
            "ones1": ones1,
            "ident": ident,
        }
        if not causal:
            m["maskT"] = maskT
        in_maps.append(m)

    trace = os.environ.get("KERNEL_TRACE", "0") == "1"
    res = run_bass_kernel_spmd(nc, in_maps, list(range(8)), trace=trace)
    LAST_RESULTS = res

    out = np.empty((B, T, HID), dtype=np.float32)
    for b in range(B):
        acc = res.results[4 * b]["out_part"].astype(np.float32)
        for g in range(1, 4):
            acc += res.results[4 * b + g]["out_part"].astype(np.float32)
        out[b] = acc
    return out


# revision 21
# speedup vs baseline: 1.1480x; 1.0276x over previous
"""DecoderAttention (GQA + RoPE + causal) Trainium2 Bass kernel.

Sharding over 8 NeuronCores: core = 4*batch + g where g in [0,4) is the
head-group. Each core computes 4 query heads (o-slice 512g:512g+512 of Wq)
and their shared KV head (slice 128g:128g+128 of Wk/Wv), plus the partial
output projection with the matching 512-column slice of Wo. Host sums the 4
partials per batch.

Per-core dataflow (matmul inputs bf16, f32 PSUM accumulate):
  QT[o,t] = WqT.T @ hsT   (transposed projections; hsT streamed once)
  RoPE applied in [d,t] layout via partition-offset DVE ops (bf16)
  ST[k,q] = KT_tile.T @ QT  -> exp on ACT (scale folded) -> P[k,q]
  attn_outT[d,q] += V_tile.T @ P ; rowsum[1,q] += ones.T @ P
  normalize at the PSUM->SBUF copy; out[t,h] += ao_tile.T @ WoT
Causal structure: fully-masked k-tiles skipped; diagonal-tile score matmuls,
exp, rowsum and AV matmuls are all column-sliced to the valid q range.
"""
import math
import os
import sys

sys.path.insert(0, "/opt/trn_rl_repo")

import numpy as np
import ml_dtypes

import concourse.bass as bass  # noqa: F401  (registers engines)
import concourse.mybir as mybir
import concourse.tile as tile
from concourse import bacc
from concourse.bass_utils import run_bass_kernel_spmd

B, T, HID = 2, 2048, 2048
H, KVH, D = 16, 4, 128
NH = H // KVH          # q-heads per core = 4
TM = 512               # t/q macro tile
NKT = HID // 128       # 16 contraction k-tiles for projections
NTT = T // 128         # 16 t-tiles
NM = T // TM           # 4 macros
SCALE = 1.0 / math.sqrt(D)
NEG = -1.0e30

f32 = mybir.dt.float32
bf16 = mybir.dt.bfloat16
fp8 = mybir.dt.float8e4
NP_IN = ml_dtypes.bfloat16
NP_F8 = ml_dtypes.float8_e4m3
DR = mybir.MatmulPerfMode.DoubleRow
WS = 32.0               # host pre-scale on Wq/Wk/Wv for the fp8 hi/lo split
FP8MAX = 240.0
EXP = mybir.ActivationFunctionType.Exp
IDENT = mybir.ActivationFunctionType.Identity
MULT = mybir.AluOpType.mult
ADD = mybir.AluOpType.add

LAST_RESULTS = None  # BassKernelResults of the most recent run (for test.py)

_cache = {}


def _emit(nc, tc, causal):
    ap = {}
    ap["hsT"] = nc.dram_tensor("hsT", [HID, T], bf16, kind="ExternalInput").ap()
    ap["wqkvT"] = nc.dram_tensor("wqkvT", [HID, 768], bf16, kind="ExternalInput").ap()
    ap["woT"] = nc.dram_tensor("woT", [512, HID], bf16, kind="ExternalInput").ap()
    ap["bias"] = nc.dram_tensor("bias", [128, 6], f32, kind="ExternalInput").ap()
    ap["cosT"] = nc.dram_tensor("cosT", [D, T], bf16, kind="ExternalInput").ap()
    ap["sinTs"] = nc.dram_tensor("sinTs", [D, T], bf16, kind="ExternalInput").ap()
    ap["dmask"] = nc.dram_tensor("dmask", [4, 128, 128], bf16, kind="ExternalInput").ap()
    ap["ones1"] = nc.dram_tensor("ones1", [128, 1], bf16, kind="ExternalInput").ap()
    ap["ident"] = nc.dram_tensor("ident", [128, 128], bf16, kind="ExternalInput").ap()
    if not causal:
        ap["maskT"] = nc.dram_tensor("maskT", [T, T], f32, kind="ExternalInput").ap()
    out_part = nc.dram_tensor("out_part", [T, HID], bf16, kind="ExternalOutput").ap()

    with tc.tile_pool(name="persist", bufs=1) as pper, \
         tc.tile_pool(name="wqkv", bufs=1) as pw, \
         tc.tile_pool(name="wo", bufs=1) as pwo, \
         tc.tile_pool(name="ropecs", bufs=1) as pcs, \
         tc.tile_pool(name="phA", bufs=2) as pa, \
         tc.tile_pool(name="hst", bufs=16) as ph, \
         tc.tile_pool(name="ptile", bufs=18) as pp, \
         tc.tile_pool(name="phB", bufs=2) as pb, \
         tc.tile_pool(name="mask", bufs=3) as pm, \
         tc.tile_pool(name="outp", bufs=4) as po:
        qt = [pper.tile([128, T], bf16, tag=f"qt{h}", name=f"qt{h}") for h in range(NH)]
        kt = pper.tile([128, T], bf16, tag="kt", name="kt")
        vsb = pper.tile([128, T], bf16, tag="vsb", name="vsb")
        ao = [pper.tile([128, T], bf16, tag=f"ao{h}", name=f"ao{h}") for h in range(NH)]
        # Matmul-critical DMAs first, interleaved across the sync and gpsimd
        # queues so the first macro's hsT tiles are all prefetched without
        # gating on PE progress. Small constant loads go afterwards.
        w_tiles = []
        pre_ht = []
        for k in range(NKT):
            wt = pw.tile([128, 768], bf16, tag=f"w{k}", name=f"w{k}")
            nc.sync.dma_start(out=wt[:], in_=ap["wqkvT"][128 * k:128 * (k + 1), :])
            w_tiles.append(wt)
            h_t = ph.tile([128, TM], bf16, tag="hst", name="hst")
            eng = nc.gpsimd if (k % 2 == 0) else nc.sync
            eng.dma_start(out=h_t[:], in_=ap["hsT"][128 * k:128 * (k + 1), 0:TM])
            pre_ht.append(h_t)
        bias_t = pper.tile([128, 6], f32, tag="bias", name="bias")
        nc.gpsimd.dma_start(out=bias_t[:], in_=ap["bias"][:])
        ones_r = pper.tile([128, 1], bf16, tag="ones1", name="ones1")
        nc.gpsimd.dma_start(out=ones_r[:], in_=ap["ones1"][:])
        ident_t = pper.tile([128, 128], bf16, tag="ident", name="ident")
        nc.gpsimd.dma_start(out=ident_t[:], in_=ap["ident"][:])
        dmask_t = pper.tile([128, 4 * 128], bf16, tag="dmask", name="dmask")
        for j in range(4):
            nc.gpsimd.dma_start(
                out=dmask_t[:, 128 * j:128 * (j + 1)], in_=ap["dmask"][j])
        cos_t = pcs.tile([128, T], bf16, tag="cosT", name="cosT")
        nc.gpsimd.dma_start(out=cos_t[:], in_=ap["cosT"][:])
        sins_t = pcs.tile([128, T], bf16, tag="sinTs", name="sinTs")
        nc.gpsimd.dma_start(out=sins_t[:], in_=ap["sinTs"][:])
        wo_tiles = [pwo.tile([128, HID], bf16, tag=f"wo{o}", name=f"wo{o}")
                    for o in range(4)]

        def load_wo():
            for o in range(4):
                nc.sync.dma_start(
                    out=wo_tiles[o][:], in_=ap["woT"][128 * o:128 * (o + 1), :])

        # ---------------- Phase A: projections + RoPE + V transpose ---------
        with tc.tile_pool(name="psA", bufs=1, space="PSUM") as psa, \
             tc.tile_pool(name="psAtr", bufs=2, space="PSUM") as psatr:

            def rope(dst, src, tsl):
                # dst = src*cos + rotate_half(src)*sin, in [d, t] layout, bf16.
                # sins_t rows d<64 hold +sin[d+64], rows d>=64 hold -sin[d-64],
                # so each mul reads both SBUF inputs at the same base partition
                # (walrus requires equal input base partitions); only the
                # output is partition-shifted.
                tmp = pa.tile([128, TM], bf16, tag="ropetmp", name="ropetmp")
                nc.vector.tensor_tensor(
                    out=tmp[0:64, :], in0=src[64:128, :], in1=sins_t[64:128, tsl], op=MULT)
                nc.vector.tensor_tensor(
                    out=tmp[64:128, :], in0=src[0:64, :], in1=sins_t[0:64, tsl], op=MULT)
                tmp2 = pa.tile([128, TM], bf16, tag="ropetmp2", name="ropetmp2")
                nc.vector.tensor_tensor(
                    out=tmp2[:], in0=src[:], in1=cos_t[:, tsl], op=MULT)
                nc.gpsimd.tensor_tensor(out=dst, in0=tmp2[:], in1=tmp[:], op=ADD)

            vraw_prev = None

            def v_transposes(m, vraw):
                for j in range(4):
                    tt = 4 * m + j
                    tr_ps = psatr.tile([128, 128], bf16, tag="vtr", name="vtr")
                    nc.tensor.transpose(
                        tr_ps[:], vraw[:, 128 * j:128 * (j + 1)], ident_t[:])
                    nc.scalar.copy(vsb[:, 128 * tt:128 * (tt + 1)], tr_ps[:])

            for m in range(NM):
                tsl = slice(TM * m, TM * (m + 1))
                q_ps = [psa.tile([128, TM], f32, tag=f"psq{o}", name=f"psq{o}")
                        for o in range(NH)]
                k_ps = psa.tile([128, TM], f32, tag="psk", name="psk")
                v_ps = psa.tile([128, TM], f32, tag="psv", name="psv")
                for k in range(NKT):
                    if m == 0:
                        h_t = pre_ht[k]
                    else:
                        h_t = ph.tile([128, TM], bf16, tag="hst", name="hst")
                        eng = nc.gpsimd if (k % 2 == 0) else nc.sync
                        eng.dma_start(
                            out=h_t[:], in_=ap["hsT"][128 * k:128 * (k + 1), tsl])
                    st = (k == 0)
                    sp = (k == NKT - 1)
                    for o in range(NH):
                        nc.tensor.matmul(
                            q_ps[o][:], w_tiles[k][:, 128 * o:128 * (o + 1)], h_t[:],
                            start=st, stop=sp)
                    nc.tensor.matmul(
                        k_ps[:], w_tiles[k][:, 512:640], h_t[:], start=st, stop=sp)
                    nc.tensor.matmul(
                        v_ps[:], w_tiles[k][:, 640:768], h_t[:], start=st, stop=sp)
                # previous macro's V transposes: their inputs are long since
                # ready, so they never stall the PE at the macro boundary.
                if vraw_prev is not None:
                    v_transposes(m - 1, vraw_prev)
                # drain the six accumulators on ACT (idle during phase A,
                # keeping DVE/Pool free for the rope chain)
                raws = []
                for o in range(NH):
                    qraw = pa.tile([128, TM], bf16, tag=f"qraw{o}", name=f"qraw{o}")
                    nc.scalar.activation(
                        qraw[:], q_ps[o][:], IDENT, bias=bias_t[:, o:o + 1])
                    raws.append(qraw)
                kraw = pa.tile([128, TM], bf16, tag="kraw", name="kraw")
                nc.scalar.activation(kraw[:], k_ps[:], IDENT, bias=bias_t[:, 4:5])
                for o in range(NH):
                    rope(qt[o][:, tsl], raws[o], tsl)
                rope(kt[:, tsl], kraw, tsl)
                vraw = pa.tile([128, TM], bf16, tag="vraw", name="vraw", bufs=2)
                nc.scalar.activation(vraw[:], v_ps[:], IDENT, bias=bias_t[:, 5:6])
                vraw_prev = vraw
                if m == 1:
                    load_wo()
            v_transposes(NM - 1, vraw_prev)

        # ---------------- Phase B + C: attention + output projection --------
        with tc.tile_pool(name="psSC", bufs=2, space="PSUM") as ps_sc, \
             tc.tile_pool(name="psAV", bufs=1, space="PSUM") as ps_av, \
             tc.tile_pool(name="psRS", bufs=1, space="PSUM") as ps_rs:
            otidx = 0
            for m in (1, 2, 3, 0):
                nk = 4 * (m + 1) if causal else NTT
                qsl = slice(TM * m, TM * (m + 1))
                for pair in range(NH // 2):
                    h0, h1 = 2 * pair, 2 * pair + 1
                    av = [ps_av.tile([128, TM], f32, tag=f"av{i}", name=f"av{i}")
                          for i in range(2)]
                    rs = [ps_rs.tile([1, TM], f32, tag=f"rs{i}", name=f"rs{i}")
                          for i in range(2)]
                    # Software pipeline: scores+exp for all visits first; then
                    # rowsums; the reciprocal/broadcast chain overlaps the AV
                    # matmuls; each head's normalize overlaps the other head's
                    # AV streams.
                    pt_q = []

                    def _q0(kk):
                        # first q column this visit contributes to (causal):
                        # q_local < 128*jp is entirely masked, never read
                        jp = kk - (nk - 4)
                        return 128 * jp if (causal and jp > 0) else 0

                    def rs_mms(kk, pt):
                        q0 = _q0(kk)
                        st = (kk == 0)
                        sp = (kk == nk - 1)
                        nc.tensor.matmul(rs[0][:, q0:TM], ones_r[:],
                                         pt[:, 0, q0:TM], start=st, stop=sp)
                        nc.tensor.matmul(rs[1][:, q0:TM], ones_r[:],
                                         pt[:, 1, q0:TM], start=st, stop=sp)

                    def av_mms(kk, pt, i):
                        q0 = _q0(kk)
                        ksl = slice(128 * kk, 128 * (kk + 1))
                        st = (kk == 0)
                        sp = (kk == nk - 1)
                        nc.tensor.matmul(av[i][:, q0:TM], vsb[:, ksl],
                                         pt[:, i, q0:TM], start=st, stop=sp)

                    for kk in range(nk):
                        ksl = slice(128 * kk, 128 * (kk + 1))
                        q0 = _q0(kk)
                        # both heads' score tiles side by side -> one exp pass
                        sc = ps_sc.tile([128, 2, TM], f32, tag="sc", name="sc")
                        nc.tensor.matmul(sc[:, 0, q0:TM], kt[:, ksl],
                                         qt[h0][:, TM * m + q0:TM * (m + 1)],
                                         start=True, stop=True)
                        nc.tensor.matmul(sc[:, 1, q0:TM], kt[:, ksl],
                                         qt[h1][:, TM * m + q0:TM * (m + 1)],
                                         start=True, stop=True)
                        pt = pp.tile([128, 2, TM], bf16, tag="pt", name="pt")
                        if causal:
                            nc.scalar.activation(
                                pt[:, :, q0:TM], sc[:, :, q0:TM], EXP, scale=SCALE)
                            jp = kk - (nk - 4)
                            if jp >= 0:
                                # mask after exp: only the 128x128 diagonal
                                # triangle is ever read partially masked (the
                                # region left of it is skipped by the sliced
                                # AV/rowsum matmuls)
                                w0, w1 = 128 * jp, 128 * (jp + 1)
                                for i in range(2):
                                    nc.vector.tensor_tensor(
                                        out=pt[:, i, w0:w1],
                                        in0=pt[:, i, w0:w1],
                                        in1=dmask_t[:, 128 * jp:128 * (jp + 1)],
                                        op=MULT)
                        else:
                            mk = pm.tile([128, TM], f32, tag="mk", name="mk")
                            nc.sync.dma_start(
                                out=mk[:], in_=ap["maskT"][ksl, qsl])
                            for i in range(2):
                                nc.vector.scalar_tensor_tensor(
                                    out=sc[:, i, :],
                                    in0=sc[:, i, :],
                                    scalar=SCALE, in1=mk[:],
                                    op0=MULT, op1=ADD)
                            nc.scalar.activation(pt[:], sc[:], EXP, scale=1.0)
                        pt_q.append((kk, pt))
                    # rowsum matmuls first so the reciprocal/broadcast chain
                    # runs concurrently with the AV matmuls.
                    for kk2, pt2 in pt_q:
                        rs_mms(kk2, pt2)
                    invbs = []
                    for i in range(2):
                        inv = pb.tile([1, TM], f32, tag="inv", name="inv")
                        nc.vector.reciprocal_approx_fast(out=inv[:], in_=rs[i][:])
                        invb = pb.tile([128, TM], f32, tag=f"invb{i}",
                                       name=f"invb{i}")
                        nc.gpsimd.partition_broadcast(invb[:], inv[:])
                        invbs.append(invb)
                    # head 0's AV streams, then its normalize overlaps head 1's
                    # AV streams, so only normalize(h1) sits on the critical
                    # path into phase C.
                    for i, h in ((0, h0), (1, h1)):
                        for kk2, pt2 in pt_q:
                            av_mms(kk2, pt2, i)
                        nc.vector.tensor_tensor(
                            out=ao[h][:, qsl], in0=av[i][:], in1=invbs[i][:],
                            op=MULT)
                    pt_q.clear()
                # Phase C for the 4 t-tiles of this macro
                for j in range(4):
                    tt = 4 * m + j
                    ttsl = slice(128 * tt, 128 * (tt + 1))
                    # software pipeline: the o<3 partial sums for all 4 output
                    # columns first, so the last head's normalize (which only
                    # lands right before this) is covered by 12 matmuls of
                    # independent work before the o=3 contributions need it.
                    units = []
                    for hc in range(4):
                        hsl = slice(512 * hc, 512 * (hc + 1))
                        if hc < 2:
                            op_ps = ps_av.tile([128, TM], f32, tag=f"av{hc}",
                                               name="opps")
                        else:
                            op_ps = ps_rs.tile([128, TM], f32, tag=f"rs{hc - 2}",
                                               name="opps")
                        for o in range(3):
                            nc.tensor.matmul(
                                op_ps[:], ao[o][:, ttsl], wo_tiles[o][:, hsl],
                                start=(o == 0), stop=False)
                        units.append((op_ps, hsl))
                    for op_ps, hsl in units:
                        nc.tensor.matmul(
                            op_ps[:], ao[3][:, ttsl], wo_tiles[3][:, hsl],
                            start=False, stop=True)
                        ot = po.tile([128, TM], bf16, tag="ot", name="ot")
                        otidx += 1
                        if otidx % 2 == 0:
                            nc.scalar.copy(ot[:], op_ps[:])
                        else:
                            nc.vector.tensor_copy(ot[:], op_ps[:])
                        nc.sync.dma_start(out=out_part[ttsl, hsl], in_=ot[:])


def _build(causal):
    nc = bacc.Bacc("TRN2", target_bir_lowering=False, debug=False, num_devices=8)
    with tile.TileContext(nc) as tc:
        _emit(nc, tc, causal)
    nc.compile()
    return nc


def _canonical_causal_mask():
    neg = np.float32(np.finfo(np.float32).min)
    m = np.where(np.tril(np.ones((T, T), dtype=bool)), np.float32(0.0), neg)
    return m.astype(np.float32)


def kernel(**inputs):
    global LAST_RESULTS
    hs = np.ascontiguousarray(np.asarray(inputs["hidden_states"], dtype=np.float32))
    cos = np.asarray(inputs["cos"], dtype=np.float32)
    sin = np.asarray(inputs["sin"], dtype=np.float32)
    mask = np.asarray(inputs["attention_mask"], dtype=np.float32)
    Wq = np.asarray(inputs["Wq"], dtype=np.float32)
    Wk = np.asarray(inputs["Wk"], dtype=np.float32)
    Wv = np.asarray(inputs["Wv"], dtype=np.float32)
    Wo = np.asarray(inputs["Wo"], dtype=np.float32)
    bq = np.asarray(inputs["bq"], dtype=np.float32)
    bk = np.asarray(inputs["bk"], dtype=np.float32)
    bv = np.asarray(inputs["bv"], dtype=np.float32)

    causal = bool(np.array_equal(mask[0, 0], _canonical_causal_mask()))

    key = (causal,)
    if key not in _cache:
        _cache[key] = _build(causal)
    nc = _cache[key]

    tri01 = (np.arange(128)[:, None] <= np.arange(128)[None, :])
    dmask = np.broadcast_to(tri01.astype(NP_IN), (4, 128, 128)).copy()
    ident = np.eye(128, dtype=NP_IN)
    ones1 = np.ones((128, 1), dtype=NP_IN)
    if not causal:
        maskT = np.ascontiguousarray(mask[0, 0].T)

    in_maps = []
    for c in range(8):
        b, g = divmod(c, 4)
        sl_q = slice(512 * g, 512 * (g + 1))
        sl_kv = slice(128 * g, 128 * (g + 1))
        sinT = np.ascontiguousarray(sin[b].T)  # [D, T]
        # row d<64: +sin[d+64] (consumed at base partition 0 writing rows 64:128)
        # row d>=64: -sin[d-64] (consumed at base partition 64 writing rows 0:64)
        sinTs = np.concatenate([sinT[64:128], -sinT[0:64]], axis=0)
        bias = np.zeros((128, 6), dtype=np.float32)
        bias[:, 0:4] = bq[sl_q].reshape(4, 128).T
        bias[:, 4] = bk[sl_kv]
        bias[:, 5] = bv[sl_kv]
        m = {
            "hsT": np.ascontiguousarray(hs[b].T.astype(NP_IN)),
            "wqkvT": np.ascontiguousarray(
                np.concatenate([Wq[sl_q], Wk[sl_kv], Wv[sl_kv]], axis=0).T.astype(NP_IN)),
            "woT": np.ascontiguousarray(Wo[:, sl_q].T.astype(NP_IN)),
            "bias": bias,
            "cosT": np.ascontiguousarray(cos[b].T.astype(NP_IN)),
            "sinTs": np.ascontiguousarray(sinTs.astype(NP_IN)),
            "dmask": dmask,
            "ones1": ones1,
            "ident": ident,
        }
        if not causal:
            m["maskT"] = maskT
        in_maps.append(m)

    trace = os.environ.get("KERNEL_TRACE", "0") == "1"
    res = run_bass_kernel_spmd(nc, in_maps, list(range(8)), trace=trace)
    LAST_RESULTS = res

    out = np.empty((B, T, HID), dtype=np.float32)
    for b in range(B):
        acc = res.results[4 * b]["out_part"].astype(np.float32)
        for g in range(1, 4):
            acc += res.results[4 * b + g]["out_part"].astype(np.float32)
        out[b] = acc
    return out
